# revision 21
# baseline (speedup 1.0000x reference)
"""AdvancedNeuroplasticityLayer — Trainium2 Bass kernel (8-core SPMD).

Reference math (B=128, I=2048, O=2048, SEG=10, all fp32):
    astro_mod = sigmoid(astrocyte_activation * context)            # [O]
    dend      = sum_j relu(einsum('bi,oij->boj', x, DS))           # [B, O]
    out       = x @ (weight * astro_mod[:,None]).T + bias + dend   # [B, O]

Distribution: tensor-parallel shard of the output dim O across the 8
NeuronCores (O_SH = 256 per core); host concatenates the slices.

Numerics tricks (all error-budgeted against the 2e-2 absmax-rel gate;
measured rel err 1.58e-2 on the reference inputs):

1. abs-decomposition: relu(y) = (y + |y|)/2.  The exact linear half is
   folded on the host into the gated weight
       w' = weight * astro_mod[:, None] + 0.5 * DS.sum(-1)
   so the device only needs 0.5*|y_j| (ScalarE Abs instead of Relu).

2. fp8 dendrite stream, split by segment for speed vs accuracy:
   - segments 0-4: float8_e4m3 (x64), consumed by DoubleRow matmuls
     (2 fp8 weights/PE cell, 0.5 cycles/row -> 4x fp16 throughput).
     Stationary operand is x8 = e4m3(x), built on device by ScalarE.
     Verified on HW: DoubleRow computes
     out[m,n] = sum_p sum_j lhsT[p,j,m] * rhs[p,j,n]  (k-pairs).
   - segments 5-9: float8_e3m4 (x64, 4 mantissa bits), classic matmuls
     against the fp16 x^T stationary.
   PSUM holds 64*y; ScalarE computes |psum/128| = 0.5|y|.

3. wg = w'^T in float8_e3m4 (x64), dequantized through the stationary
   side: xs = x/64 in fp16 built on device, so psum_w = x @ w'.T exactly.

4. bias is applied on the host after the gather (all-zeros here anyway).

Groups pair by o-range so the segment reduction pipelines under the
stream: for each of the o-ranges [0,102) [102,204) [204,256), one
DoubleRow group (j<5) and one classic group (j>=5) share an o-aligned
column block (510/510/260 columns), written into the common
dr[128, o, j] layout by strided Abs; reduce chunk c fires as soon as
the pair covering its o-range is done.

Cost model: DMA 6.42 MB @ 360 GB/s = 17.8 us; PE ~12.6 us (5.1k cycles
DoubleRow dendrite + 20.5k classic + 4.1k weight) -> stream-bound.
DMA order: interleaved x/ds head, group pairs, wg last in two column
halves with pipelined add+store tails.
"""

import numpy as np
import ml_dtypes

import concourse.bass as bass  # noqa: F401  (bass types referenced via bacc)
import concourse.tile as tile
from concourse import bacc, mybir
from concourse import bass_utils

B, I, O, SEG = 128, 2048, 2048, 10
NCORES = 8
O_SH = O // NCORES            # 256 output columns per core
KT = I // 128                 # 16 contraction tiles
KH = KT // 2                  # 8 k-tiles per half DMA
NT = KT // 2                  # 8 DoubleRow k-pairs
SEG_DR = 5                    # segments 0-4 via DoubleRow e4m3
SEG_CL = SEG - SEG_DR         # segments 5-9 via classic e3m4

# o-ranges per group pair (o-aligned, <=512 psum columns per group)
O_RANGES = [(0, 102), (102, 204), (204, 256)]

F16 = mybir.dt.float16
F32 = mybir.dt.float32
F8E3 = mybir.dt.float8e3
F8E4 = mybir.dt.float8e4

NP_F16 = np.float16
NP_E3M4 = ml_dtypes.float8_e3m4
NP_E4M3 = ml_dtypes.float8_e4m3

DS_SCALE = 64.0               # pre-scale for both fp8 dendrite streams
WG_SCALE = 64.0               # pre-scale for the e3m4 gated-weight stream


def build_nc():
    nc = bacc.Bacc("TRN2", target_bir_lowering=False, debug=False)

    xT = nc.dram_tensor("xT", [128, KT * B], F16, kind="ExternalInput").ap()
    dsdr = [
        nc.dram_tensor(
            f"dsdr{i}", [128, NT * 2 * (o1 - o0) * SEG_DR], F8E4,
            kind="ExternalInput",
        ).ap()
        for i, (o0, o1) in enumerate(O_RANGES)
    ]
    dscl = [
        nc.dram_tensor(
            f"dscl{i}", [128, KT * (o1 - o0) * SEG_CL], F8E3,
            kind="ExternalInput",
        ).ap()
        for i, (o0, o1) in enumerate(O_RANGES)
    ]
    wg = nc.dram_tensor("wg", [128, KT * O_SH], F8E3, kind="ExternalInput").ap()
    out = nc.dram_tensor("out", [B, O_SH], F32, kind="ExternalOutput").ap()

    with tile.TileContext(nc) as tc:
        with (
            tc.tile_pool(name="xw", bufs=1) as xwpool,
            tc.tile_pool(name="dst", bufs=1) as dspool,
            tc.tile_pool(name="dr", bufs=1) as drpool,
            tc.tile_pool(name="fin", bufs=1) as finpool,
            tc.tile_pool(name="psw", bufs=1, space="PSUM") as pswpool,
            tc.tile_pool(name="psd", bufs=1, space="PSUM") as psdpool,
        ):
            xt_flat = xwpool.tile([128, KT * B], F16)
            xs_flat = xwpool.tile([128, KT * B], F16)   # x/64 (wg stationary)
            x8_flat = xwpool.tile([128, KT * B], F8E4)  # e4m3(x) (DR stationary)
            wgt_flat = xwpool.tile([128, KT * O_SH], F8E3)

            dr = drpool.tile([128, O_SH, SEG], F32)
            dend = finpool.tile([128, O_SH], F32)
            osum = finpool.tile([128, O_SH], F32)
            psw = [pswpool.tile([128, O_SH // 2], F32, name=f"psw{h}")
                   for h in range(2)]

            xt = xt_flat[:].rearrange("p (k m) -> p k m", k=KT)
            xs = xs_flat[:].rearrange("p (k m) -> p k m", k=KT)
            x8 = x8_flat[:].rearrange("p (t j m) -> p t j m", t=NT, j=2)
            drv = dr[:]  # [128, o, j]

            OCH = 64  # o-granularity of the segment reduction

            def reduce_chunk(c):
                nc.vector.reduce_sum(
                    dend[:, c * OCH : (c + 1) * OCH],
                    drv[:, c * OCH : (c + 1) * OCH, :],
                    axis=mybir.AxisListType.X,
                )

            # ---- DMA head: x pieces interleave with the first ds pieces ----
            X_PIECES = [(0, 4), (4, 10), (10, 16)]

            def load_x(ka, kb):
                nc.sync.dma_start(
                    xt_flat[:, ka * B : kb * B], xT[:, ka * B : kb * B]
                )
                # derived stationaries, built by ScalarE under the DMA head
                nc.scalar.activation(
                    xs_flat[:, ka * B : kb * B],
                    xt_flat[:, ka * B : kb * B],
                    mybir.ActivationFunctionType.Copy,
                    scale=1.0 / WG_SCALE,
                )
                nc.scalar.activation(
                    x8_flat[:, ka * B : kb * B],
                    xt_flat[:, ka * B : kb * B],
                    mybir.ActivationFunctionType.Copy,
                )

            load_x(*X_PIECES[0])

            # ---- dendrite group pairs ----
            # Within each pair the PE-heavy classic group streams first so
            # PE ramps up early; the PE-light DoubleRow group follows.
            for gi, (o0, o1) in enumerate(O_RANGES):
                No = o1 - o0
                C5 = No * SEG_DR           # columns per group

                # classic group (segments 5-9), e3m4, k-tile pieces
                psB = psdpool.tile([128, C5], F32, name=f"psCL{gi}")
                if gi == 0:
                    cl_pieces = [(0, 4), (4, KH), (KH, KT)]
                elif gi < len(O_RANGES) - 1:
                    cl_pieces = [(0, KH), (KH, KT)]
                else:
                    cl_pieces = [(0, KH), (KH, 14), (14, KT)]
                for (k0, k1) in cl_pieces:
                    npc = (k1 - k0) * C5
                    dsg = dspool.tile([128, npc], F8E3,
                                      name=f"dscl_{gi}_{k0}")
                    nc.sync.dma_start(
                        dsg[:], dscl[gi][:, k0 * C5 : k1 * C5]
                    )
                    if gi == 0:
                        if k0 == 4:
                            load_x(*X_PIECES[1])
                        elif k0 == KH:
                            load_x(*X_PIECES[2])
                    dsgv = dsg[:].rearrange("p (k n) -> p k n", k=k1 - k0)
                    for k in range(k0, k1):
                        nc.tensor.matmul(
                            psB[:], xt[:, k, :], dsgv[:, k - k0, :],
                            start=(k == 0), stop=(k == KT - 1),
                        )
                nc.scalar.activation(
                    dr[:, o0:o1, SEG_DR:SEG], psB[:],
                    mybir.ActivationFunctionType.Abs,
                    scale=1.0 / (2.0 * DS_SCALE),
                )

                # DoubleRow group (segments 0-4), e4m3, k-pair pieces
                psA = psdpool.tile([128, C5], F32, name=f"psDR{gi}")
                for (t0, t1) in [(0, 4), (4, NT)]:
                    npc = (t1 - t0) * 2 * C5
                    dsg = dspool.tile([128, npc], F8E4,
                                      name=f"dsdr_{gi}_{t0}")
                    nc.sync.dma_start(
                        dsg[:], dsdr[gi][:, t0 * 2 * C5 : t1 * 2 * C5]
                    )
                    dsgv = dsg[:].rearrange(
                        "p (t j n) -> p t j n", t=t1 - t0, j=2
                    )
                    for t in range(t0, t1):
                        nc.tensor.matmul(
                            psA[:], x8[:, t, :, :], dsgv[:, t - t0, :, :],
                            start=(t == 0), stop=(t == NT - 1),
                            perf_mode=mybir.MatmulPerfMode.DoubleRow,
                        )
                # dr[:, o0:o1, 0:5] = |psA/128| = 0.5*|y|
                nc.scalar.activation(
                    dr[:, o0:o1, 0:SEG_DR], psA[:],
                    mybir.ActivationFunctionType.Abs,
                    scale=1.0 / (2.0 * DS_SCALE),
                )

                # reduce chunks whose o-range this pair completes
                if gi == 0:
                    reduce_chunk(0)            # o 0-63    (<=101)
                elif gi == 1:
                    reduce_chunk(1)            # o 64-127  (<=203)
                    reduce_chunk(2)            # o 128-191 (<=203)

            # ---- gated-linear weight last: shortest tail chain ----
            # column-split into two 128-col halves with pipelined add+store
            OH = O_SH // 2
            W_PIECES = [(0, 10), (10, 16)]
            for h in range(2):
                wgh = wgt_flat[:, h * KT * OH : (h + 1) * KT * OH].rearrange(
                    "p (k n) -> p k n", k=KT
                )
                for (k0, k1) in W_PIECES:
                    nc.sync.dma_start(
                        wgt_flat[:, h * KT * OH + k0 * OH :
                                 h * KT * OH + k1 * OH],
                        wg[:, h * KT * OH + k0 * OH : h * KT * OH + k1 * OH],
                    )
                    for k in range(k0, k1):
                        nc.tensor.matmul(
                            psw[h][:], xs[:, k, :], wgh[:, k, :],
                            start=(k == 0), stop=(k == KT - 1),
                        )
                if h == 0:
                    reduce_chunk(3)            # o 192-255, after pair 2
                nc.vector.tensor_add(
                    osum[:, h * OH : (h + 1) * OH],
                    dend[:, h * OH : (h + 1) * OH],
                    psw[h][:],
                )
                nc.sync.dma_start(
                    out[:, h * OH : (h + 1) * OH],
                    osum[:, h * OH : (h + 1) * OH],
                )

    nc.compile()
    return nc


def prep_inputs(x, context, prev_activation, weight, bias, astrocyte_activation,
                dendrite_segments):
    """Host-side shard + pack into the DMA-friendly per-core layouts."""
    x = np.asarray(x, dtype=np.float32)
    weight = np.asarray(weight, dtype=np.float32)
    bias = np.asarray(bias, dtype=np.float32)
    context = np.asarray(context, dtype=np.float32)
    astro = np.asarray(astrocyte_activation, dtype=np.float32)
    ds_full = np.asarray(dendrite_segments, dtype=np.float32)

    astro_mod = 1.0 / (1.0 + np.exp(-(astro * context)))
    # abs-decomposition: fold the exact linear half of the relu into the
    # gated weight (see module docstring)
    wg_full = (
        (weight * astro_mod[:, None] + 0.5 * ds_full.sum(axis=2)).T
        * WG_SCALE
    ).astype(NP_E3M4)                                             # [I, O]
    wg_k = wg_full.reshape(KT, 128, O)

    # SBUF image: xT_pack[p, k*B+m] = x[m, k*128+p]
    xT_pack = np.ascontiguousarray(
        x.reshape(B, KT, 128).transpose(2, 1, 0).reshape(128, KT * B)
    ).astype(NP_F16)

    dsT = ds_full.transpose(1, 0, 2)                              # [I, O, SEG]

    in_maps = []
    for c in range(NCORES):
        sl = slice(c * O_SH, (c + 1) * O_SH)
        blk = dsT[:, sl, :] * DS_SCALE                            # [I, 256, 10]
        dr8 = blk[:, :, :SEG_DR].astype(NP_E4M3)                  # [I, 256, 5]
        cl8 = blk[:, :, SEG_DR:].astype(NP_E3M4)                  # [I, 256, 5]
        im = {"xT": xT_pack}
        for gi, (o0, o1) in enumerate(O_RANGES):
            No = o1 - o0
            # DR pack[p, t, j, c] = dr8[(2t+j)*128+p, o0 + c//5, c%5]
            g = dr8[:, o0:o1, :].reshape(NT, 2, 128, No * SEG_DR)
            im[f"dsdr{gi}"] = np.ascontiguousarray(
                g.transpose(2, 0, 1, 3)
            ).reshape(128, NT * 2 * No * SEG_DR)
            # CL pack[p, k, c] = cl8[k*128+p, o0 + c//5, c%5]
            g = cl8[:, o0:o1, :].reshape(KT, 128, No * SEG_CL)
            im[f"dscl{gi}"] = np.ascontiguousarray(
                g.transpose(1, 0, 2)
            ).reshape(128, KT * No * SEG_CL)
        # wg image, column-half major: for half h (128 cols each),
        # wg_pack[p, h*KT*128 + k*128 + n] = wg_k[k, p, sl][h*128 + n]
        im["wg"] = np.ascontiguousarray(
            wg_k[:, :, sl]                       # [KT, 128, 256]
            .reshape(KT, 128, 2, O_SH // 2)
            .transpose(1, 2, 0, 3)               # [128, 2, KT, 128]
            .reshape(128, KT * O_SH)
        )
        in_maps.append(im)
    return in_maps


_NC_CACHE = {}


def get_nc():
    if "nc" not in _NC_CACHE:
        _NC_CACHE["nc"] = build_nc()
    return _NC_CACHE["nc"]


def kernel(**inputs):
    nc = get_nc()
    in_maps = prep_inputs(**inputs)
    try:
        res = bass_utils.run_bass_kernel_spmd(
            nc, in_maps, core_ids=list(range(NCORES))
        )
    except Exception:
        # one retry: transient accelerator-worker failures
        # (NRT_EXEC_UNIT_UNRECOVERABLE) have been observed to recover
        res = bass_utils.run_bass_kernel_spmd(
            nc, in_maps, core_ids=list(range(NCORES))
        )
    out = np.concatenate(
        [res.results[c]["out"] for c in range(NCORES)], axis=1
    )
    # bias is applied on the host (exact; all-zeros for the reference inputs)
    return out + np.asarray(inputs["bias"], dtype=np.float32)[None, :]
